# revision 31
# baseline (speedup 1.0000x reference)
"""MoE (8 routed experts top-2 + shared expert) Trainium2 kernel, v12:
true expert-parallel with host-side dispatch.

Sharding (8 cores): core c owns
  - routed expert e = c: the host computes the (cheap, 2048x1024x8) gating
    on CPU, gathers the tokens routed to expert e into a dense [1024, 512]
    slab (seed-0 per-expert counts are 468..551; the few slots beyond
    CAP=512 fall back to an exact numpy path on the host), and the device
    runs the expert SwiGLU on the gathered slab.
  - shared-expert shard (t, g), t = c // 2 (512-token quarter), g = c % 2
    (d_expert half: columns [512g : 512g+512] of Wg_s/Wu_s, rows of Wd_s).

The device program is a pure bf16 GEMM pipeline (no gating, no gather, no
transposes on the PE): 192 N=512 matmuls per core that run back-to-back at
~217-226ns each (the 2.4GHz issue floor):
  8 up/gate phases (shared hc0..3, routed hc0..3), each = 16 accumulating
  matmuls (gate/up interleaved) into a 4-bank PSUM pair rotation, silu on
  ScalarE * up on VectorE; then 16 interleaved down chunks (shared/routed)
  through a 4-bank PSUM double-buffer, copied out by VectorE/ScalarE and
  DMA'd on both HWDGE FIFOs; the final chunk is split in half across
  engines/queues to shorten the tail.

Schedule notes (measured on HW):
  - ~7.2us framework preamble before any instruction can issue, then
    ~4.5us first-DMA latency: real work can start ~11.8us in at best.
  - All inputs ride ONE ordered sync-FIFO stream (two concurrent queues
    split bandwidth round-robin and break the global arrival order).  The
    first ~5MB must land in consumption order because the PE catches up
    with the stream during phases sh0-sh1.
  - The warmup block (24 independent N=128 matmuls, ~46% PE duty) bridges
    the preamble->data window.  Deliberately NOT a dense chain: a
    100%-duty warmup reproducibly trips the P0 power downclock and the
    whole kernel then runs at 2.0GHz instead of 2.4 (+9us).  HAM reaches
    K=8/8 ~2us into the first phase; the small cold sliver is hidden
    behind the DMA stream anyway.

Host combine: shared halves summed pairwise per token quarter; routed slot
columns scaled by the top-2 softmax weight and scatter-added.
"""

import sys

sys.path.insert(0, "/opt/trn_rl_repo")

import numpy as np
import ml_dtypes

import concourse.bass as bass
import concourse.tile as tile
import concourse.mybir as mybir
from concourse import bacc
from concourse.bass_utils import run_bass_kernel_spmd

F32 = mybir.dt.float32
BF16 = mybir.dt.bfloat16
ACT = mybir.ActivationFunctionType
ALU = mybir.AluOpType
BF = ml_dtypes.bfloat16

N_CORES = 8
D = 1024          # d_hidden
DE = 512          # d_expert (routed); also the shared-expert half width
E = 8             # routed experts
DC = D // 128     # 8 contraction chunks of 128
HC = DE // 128    # 4 expert-width chunks of 128
NT = 512          # shared tokens per core (quarter)
CAP = 512         # routed slots per core (seed-0 max expert count is 551)
N_WARM = 24       # PE warmup matmuls; see comment at the warmup block


def build_program():
    nc = bacc.Bacc(num_devices=N_CORES)

    # ---- per-core DRAM I/O (host-prearranged layouts) ----
    # x slabs: [p, dc, n] with row d = dc*128 + p on partitions
    xs_d = nc.dram_tensor("xs", [128, DC, NT], BF16, kind="ExternalInput")
    xg_d = nc.dram_tensor("xg", [128, DC, CAP], BF16, kind="ExternalInput")
    # up/gate weights: [p, hc, dc*128 + j] (hc-major so per-hc slices are
    # single contiguous DMAs)
    wsg_d = nc.dram_tensor("wsg", [128, HC, D], BF16, kind="ExternalInput")
    wsu_d = nc.dram_tensor("wsu", [128, HC, D], BF16, kind="ExternalInput")
    wg_d = nc.dram_tensor("wg", [128, HC, D], BF16, kind="ExternalInput")
    wu_d = nc.dram_tensor("wu", [128, HC, D], BF16, kind="ExternalInput")
    # down weights: [p, hc, i] with contraction row (hc*128 + p)
    wsd_d = nc.dram_tensor("wsd", [128, HC, D], BF16, kind="ExternalInput")
    wd_d = nc.dram_tensor("wd", [128, HC, D], BF16, kind="ExternalInput")

    out_sh = nc.dram_tensor("out_sh", [DC, 128, NT], BF16, kind="ExternalOutput")
    out_rt = nc.dram_tensor("out_rt", [DC, 128, CAP], BF16, kind="ExternalOutput")

    with tile.TileContext(nc) as tc:
        with (
            tc.tile_pool(name="const", bufs=1) as constp,
            tc.tile_pool(name="inp", bufs=1) as inp,
            tc.tile_pool(name="hp", bufs=1) as hp,
            tc.tile_pool(name="sp", bufs=2) as sp,
            tc.tile_pool(name="op", bufs=2) as op,
            tc.tile_pool(name="psug", bufs=1, space="PSUM") as psug,
            tc.tile_pool(name="psdn", bufs=1, space="PSUM") as psdn,
        ):
            # zeros tile for PE warmup (DVE memset starts fast; values don't
            # matter for HAM, only PE busy-ness)
            wz = constp.tile([128, 256], BF16, tag="wz")
            nc.vector.memset(wz[:], 0.0)

            # ---- input loads ----
            wsg_sb = inp.tile([128, HC, D], BF16, tag="wsg")
            wsu_sb = inp.tile([128, HC, D], BF16, tag="wsu")
            xs_sb = inp.tile([128, DC, NT], BF16, tag="xs")
            xg_sb = inp.tile([128, DC, CAP], BF16, tag="xg")
            # A single queue gets the full ~360GB/s and drains strictly in
            # order, so the stream is laid out in exact consumption order:
            # sh0 is paced by the xs quarters interleaved with its weights,
            # then each later phase's weights land just ahead of its start.
            # (Two queues split bandwidth round-robin and break the global
            # order -- measured slower.)
            wg_sb = inp.tile([128, HC, D], BF16, tag="wg")
            wu_sb = inp.tile([128, HC, D], BF16, tag="wu")
            wsd_sb = inp.tile([128, HC, D], BF16, tag="wsd")
            wd_sb = inp.tile([128, HC, D], BF16, tag="wd")
            nc.sync.dma_start(wsg_sb[:, 0, :], wsg_d[:, 0, :])
            nc.sync.dma_start(xs_sb[:, 0:2, :], xs_d[:, 0:2, :])
            nc.sync.dma_start(wsu_sb[:, 0, :], wsu_d[:, 0, :])
            nc.sync.dma_start(xs_sb[:, 2:4, :], xs_d[:, 2:4, :])
            nc.sync.dma_start(wsg_sb[:, 1, :], wsg_d[:, 1, :])
            nc.sync.dma_start(xs_sb[:, 4:6, :], xs_d[:, 4:6, :])
            nc.sync.dma_start(wsu_sb[:, 1, :], wsu_d[:, 1, :])
            nc.sync.dma_start(xs_sb[:, 6:8, :], xs_d[:, 6:8, :])
            for hc in range(2, HC):
                nc.sync.dma_start(wsg_sb[:, hc, :], wsg_d[:, hc, :])
                nc.sync.dma_start(wsu_sb[:, hc, :], wsu_d[:, hc, :])
            nc.sync.dma_start(xg_sb[:, 0:4, :], xg_d[:, 0:4, :])
            nc.sync.dma_start(xg_sb[:, 4:8, :], xg_d[:, 4:8, :])
            for hc in range(HC):
                nc.sync.dma_start(wg_sb[:, hc, :], wg_d[:, hc, :])
                nc.sync.dma_start(wu_sb[:, hc, :], wu_d[:, hc, :])
            nc.sync.dma_start(wsd_sb[:], wsd_d[:])
            nc.sync.dma_start(wd_sb[:], wd_d[:])

            # ---- PE p-state warmup: independent MMs alternating two banks
            # (~46% PE duty -- deliberately NOT a dense chain: a 100%-duty
            # warmup trips the P0 power downclock and the whole kernel then
            # runs at 2.0GHz instead of 2.4).  Sized to end right when the
            # first phase's operands land (~12.5us). ----
            for w in range(N_WARM):
                ps_w = psdn.tile([128, 128], F32, tag=("shA" if w % 2 == 0 else "shB"))
                nc.tensor.matmul(ps_w[:], wz[:, 0:128], wz[:, 0:128], start=True, stop=True)

            h_s = hp.tile([128, HC, NT], BF16, tag="hs")
            h_r = hp.tile([128, HC, CAP], BF16, tag="hr")

            # ---- up/gate phases: psum banks rotate over 2 tag-pairs so a
            # phase never waits on the drain of the previous one (phase k's
            # silu/mult run during phase k+1, well before phase k+2 reuses
            # the banks) ----
            def up_gate(x_sb, wgt_sb, wup_sb, n, h, hc, pair):
                ps_g = psug.tile([128, n], F32, tag=f"g{pair}")
                ps_u = psug.tile([128, n], F32, tag=f"u{pair}")
                for dc in range(DC):
                    nc.tensor.matmul(
                        ps_g[:],
                        wgt_sb[:, hc, dc * 128 : (dc + 1) * 128],
                        x_sb[:, dc, :],
                        start=(dc == 0),
                        stop=(dc == DC - 1),
                    )
                    nc.tensor.matmul(
                        ps_u[:],
                        wup_sb[:, hc, dc * 128 : (dc + 1) * 128],
                        x_sb[:, dc, :],
                        start=(dc == 0),
                        stop=(dc == DC - 1),
                    )
                sil = sp.tile([128, n], F32, tag="sil")
                nc.scalar.activation(sil[:], ps_g[:], ACT.Silu)
                nc.vector.tensor_tensor(h[:, hc, :], sil[:], ps_u[:], op=ALU.mult)

            phases = [(xs_sb, wsg_sb, wsu_sb, NT, h_s, hc) for hc in range(HC)]
            phases += [(xg_sb, wg_sb, wu_sb, CAP, h_r, hc) for hc in range(HC)]
            for k, (x_sb, wgt_sb, wup_sb, n, h, hc) in enumerate(phases):
                up_gate(x_sb, wgt_sb, wup_sb, n, h, hc, "AB"[k % 2])

            # ---- down projections, interleaved sh/rt per output chunk:
            # 4-bank PSUM double buffer; sh copies on VectorE -> sync DMA
            # FIFO, rt copies on ScalarE -> scalar DMA FIFO ----
            def down_chunk(dc, w_sb, h, n, out_d, par, otag, cp_eng, q_eng):
                ps_d = psdn.tile([128, n], F32, tag=par)
                for hc in range(HC):
                    nc.tensor.matmul(
                        ps_d[:],
                        w_sb[:, hc, dc * 128 : (dc + 1) * 128],
                        h[:, hc, :],
                        start=(hc == 0),
                        stop=(hc == HC - 1),
                    )
                o = op.tile([128, n], BF16, tag=otag, bufs=3)
                if cp_eng == "v":
                    nc.vector.tensor_copy(o[:], ps_d[:])
                else:
                    nc.scalar.activation(o[:], ps_d[:], ACT.Copy)
                q_eng.dma_start(out_d[dc], o[:])

            def down_chunk_split(dc, w_sb, h, n, out_d, otag):
                # final chunk: two independent 256-col psum groups so the
                # first half's copy+DMA overlaps the second half's matmuls,
                # and the two halves drain on both engines/queues
                for hf, (par, cp, q) in enumerate(
                    [("shA", "v", nc.sync), ("shB", "s", nc.scalar)]
                ):
                    cols = slice(hf * (n // 2), (hf + 1) * (n // 2))
                    ps_d = psdn.tile([128, n // 2], F32, tag=par)
                    for hc in range(HC):
                        nc.tensor.matmul(
                            ps_d[:],
                            w_sb[:, hc, dc * 128 : (dc + 1) * 128],
                            h[:, hc, cols],
                            start=(hc == 0),
                            stop=(hc == HC - 1),
                        )
                    o = op.tile([128, n // 2], BF16, tag=f"{otag}{hf}", bufs=2)
                    if cp == "v":
                        nc.vector.tensor_copy(o[:], ps_d[:])
                    else:
                        nc.scalar.activation(o[:], ps_d[:], ACT.Copy)
                    q.dma_start(out_d[dc, :, cols], o[:])

            for dc in range(DC):
                par = "A" if dc % 2 == 0 else "B"
                if dc < DC - 1:
                    down_chunk(dc, wsd_sb, h_s, NT, out_sh, f"sh{par}", "osh", "v", nc.sync)
                    down_chunk(dc, wd_sb, h_r, CAP, out_rt, f"rt{par}", "ort", "s", nc.scalar)
                else:
                    down_chunk_split(dc, wsd_sb, h_s, NT, out_sh, "osh7")
                    down_chunk_split(dc, wd_sb, h_r, CAP, out_rt, "ort7")

    nc.compile()
    return nc


_NC_CACHE = None


def _get_program():
    global _NC_CACHE
    if _NC_CACHE is None:
        _NC_CACHE = build_program()
    return _NC_CACHE


def _xpose_pdc(m):
    """[1024, X] -> [128, 8, X] with row (dc*128+p) at [p, dc]."""
    return np.ascontiguousarray(m.reshape(DC, 128, -1).transpose(1, 0, 2))


def _wlay_upgate(w):
    """[1024(d), 512(de)] -> [128, HC, D]: [p, hc, dc*128+j] = w[dc*128+p, hc*128+j]."""
    return np.ascontiguousarray(
        w.reshape(DC, 128, HC, 128).transpose(1, 2, 0, 3).reshape(128, HC, D)
    )


def _wlay_down(w):
    """[512(de), 1024(d)] -> [128, HC, D]: [p, hc, i] = w[hc*128+p, i]."""
    return np.ascontiguousarray(w.reshape(HC, 128, D).transpose(1, 0, 2))


def _silu(x):
    return x / (1.0 + np.exp(-x))


def kernel(x, W_g, Wg_e, Wu_e, Wd_e, Wg_s, Wu_s, Wd_s, _trace=False, _trace_kwargs=None):
    nc = _get_program()

    xf = np.asarray(x, dtype=np.float32).reshape(2 * 1024, D)
    W_g = np.asarray(W_g, dtype=np.float32)
    Wg_e = np.asarray(Wg_e, dtype=np.float32)
    Wu_e = np.asarray(Wu_e, dtype=np.float32)
    Wd_e = np.asarray(Wd_e, dtype=np.float32)
    Wg_s = np.asarray(Wg_s, dtype=np.float32)
    Wu_s = np.asarray(Wu_s, dtype=np.float32)
    Wd_s = np.asarray(Wd_s, dtype=np.float32)

    # ---- host gating (exact f32; top-2 sets match the jax reference,
    # min top2-top3 prob gap at seed 0 is 6.8e-5 >> f32 matmul noise) ----
    logits = xf @ W_g
    p = np.exp(logits - logits.max(axis=1, keepdims=True))
    p /= p.sum(axis=1, keepdims=True)                      # [N, E] softmax
    top2 = np.argsort(-p, axis=1, kind="stable")[:, :2]    # [N, 2]
    sel = np.zeros((xf.shape[0], E), dtype=bool)
    sel[np.arange(xf.shape[0])[:, None], top2] = True
    toks = [np.nonzero(sel[:, e])[0] for e in range(E)]    # ascending per expert

    # ---- per-core device inputs ----
    in_maps = []
    for c in range(N_CORES):
        e, t, g = c, c // 2, c % 2
        tl = toks[e][:CAP]
        xg = np.zeros((D, CAP), dtype=BF)
        xg[:, : len(tl)] = xf[tl].T.astype(BF)
        in_maps.append(
            {
                "xs": _xpose_pdc(xf[t * NT : (t + 1) * NT].T.astype(BF)),
                "xg": _xpose_pdc(xg),
                "wsg": _wlay_upgate(Wg_s[:, DE * g : DE * (g + 1)].astype(BF)),
                "wsu": _wlay_upgate(Wu_s[:, DE * g : DE * (g + 1)].astype(BF)),
                "wg": _wlay_upgate(Wg_e[e].astype(BF)),
                "wu": _wlay_upgate(Wu_e[e].astype(BF)),
                "wsd": _wlay_down(Wd_s[DE * g : DE * (g + 1), :].astype(BF)),
                "wd": _wlay_down(Wd_e[e].astype(BF)),
            }
        )

    res = run_bass_kernel_spmd(
        nc, in_maps, list(range(N_CORES)), trace=_trace, **(_trace_kwargs or {})
    )

    # ---- host combine (unshard) ----
    out = np.empty((2 * 1024, D), dtype=np.float32)
    for t in range(4):
        sh = np.asarray(res.results[2 * t]["out_sh"], dtype=np.float32).reshape(D, NT)
        sh += np.asarray(res.results[2 * t + 1]["out_sh"], dtype=np.float32).reshape(
            D, NT
        )
        out[t * NT : (t + 1) * NT, :] = sh.T
    for e in range(E):
        tl = toks[e]
        nd = min(len(tl), CAP)
        rt = np.asarray(res.results[e]["out_rt"], dtype=np.float32).reshape(D, CAP)
        out[tl[:nd]] += (rt[:, :nd] * p[tl[:nd], e][None, :]).T
        if len(tl) > CAP:  # over-capacity tokens: exact host fallback
            to = tl[CAP:]
            xo = xf[to]
            hh = _silu(xo @ Wg_e[e]) * (xo @ Wu_e[e])
            out[to] += (hh @ Wd_e[e]) * p[to, e][:, None]

    result = out.reshape(2, 1024, D)
    if _trace:
        return result, res
    return result


# revision 33
# speedup vs baseline: 1.0466x; 1.0466x over previous
"""MoE (8 routed experts top-2 + shared expert) Trainium2 kernel, v12:
true expert-parallel with host-side dispatch.

Sharding (8 cores): core c owns
  - routed expert e = c: the host computes the (cheap, 2048x1024x8) gating
    on CPU, gathers the tokens routed to expert e into a dense [1024, 512]
    slab (seed-0 per-expert counts are 468..551; the few slots beyond
    CAP=512 fall back to an exact numpy path on the host), and the device
    runs the expert SwiGLU on the gathered slab.
  - shared-expert shard (t, g), t = c // 2 (512-token quarter), g = c % 2
    (d_expert half: columns [512g : 512g+512] of Wg_s/Wu_s, rows of Wd_s).

The device program is a pure bf16 GEMM pipeline (no gating, no gather, no
transposes on the PE): 192 N=512 matmuls per core that run back-to-back at
~217-226ns each (the 2.4GHz issue floor):
  8 up/gate phases (shared hc0..3, routed hc0..3), each = 16 accumulating
  matmuls (gate/up interleaved) into a 4-bank PSUM pair rotation, silu on
  ScalarE * up on VectorE; then 16 interleaved down chunks (shared/routed)
  through a 4-bank PSUM double-buffer, copied out by VectorE/ScalarE and
  DMA'd on both HWDGE FIFOs; the final chunk is split in half across
  engines/queues to shorten the tail.

Schedule notes (measured on HW):
  - ~7.2us framework preamble before any instruction can issue, then
    ~4.5us first-DMA latency: real work can start ~11.8us in at best.
  - All inputs ride ONE ordered sync-FIFO stream (two concurrent queues
    split bandwidth round-robin and break the global arrival order).  The
    first ~5MB must land in consumption order because the PE catches up
    with the stream during phases sh0-sh1.
  - The warmup block (24 independent N=128 matmuls, ~46% PE duty) bridges
    the preamble->data window.  Deliberately NOT a dense chain: a
    100%-duty warmup reproducibly trips the P0 power downclock and the
    whole kernel then runs at 2.0GHz instead of 2.4 (+9us).  HAM reaches
    K=8/8 ~2us into the first phase; the small cold sliver is hidden
    behind the DMA stream anyway.

Host combine: shared halves summed pairwise per token quarter; routed slot
columns scaled by the top-2 softmax weight and scatter-added.
"""

import sys

sys.path.insert(0, "/opt/trn_rl_repo")

import numpy as np
import ml_dtypes

import concourse.bass as bass
import concourse.tile as tile
import concourse.mybir as mybir
from concourse import bacc
from concourse.bass_utils import run_bass_kernel_spmd

F32 = mybir.dt.float32
BF16 = mybir.dt.bfloat16
ACT = mybir.ActivationFunctionType
ALU = mybir.AluOpType
BF = ml_dtypes.bfloat16

N_CORES = 8
D = 1024          # d_hidden
DE = 512          # d_expert (routed); also the shared-expert half width
E = 8             # routed experts
DC = D // 128     # 8 contraction chunks of 128
HC = DE // 128    # 4 expert-width chunks of 128
NT = 512          # shared tokens per core (quarter)
CAP = 512         # routed slots per core (seed-0 max expert count is 551)
N_WARM = 24       # PE warmup matmuls; see comment at the warmup block


def build_program():
    nc = bacc.Bacc(num_devices=N_CORES)

    # ---- per-core DRAM I/O (host-prearranged layouts) ----
    # x slabs: [p, dc, n] with row d = dc*128 + p on partitions
    xs_d = nc.dram_tensor("xs", [128, DC, NT], BF16, kind="ExternalInput")
    xg_d = nc.dram_tensor("xg", [128, DC, CAP], BF16, kind="ExternalInput")
    # up/gate weights: [p, hc, dc*128 + j] (hc-major so per-hc slices are
    # single contiguous DMAs)
    wsg_d = nc.dram_tensor("wsg", [128, HC, D], BF16, kind="ExternalInput")
    wsu_d = nc.dram_tensor("wsu", [128, HC, D], BF16, kind="ExternalInput")
    wg_d = nc.dram_tensor("wg", [128, HC, D], BF16, kind="ExternalInput")
    wu_d = nc.dram_tensor("wu", [128, HC, D], BF16, kind="ExternalInput")
    # down weights: [p, hc, i] with contraction row (hc*128 + p)
    wsd_d = nc.dram_tensor("wsd", [128, HC, D], BF16, kind="ExternalInput")
    wd_d = nc.dram_tensor("wd", [128, HC, D], BF16, kind="ExternalInput")

    out_sh = nc.dram_tensor("out_sh", [DC, 128, NT], BF16, kind="ExternalOutput")
    out_rt = nc.dram_tensor("out_rt", [DC, 128, CAP], BF16, kind="ExternalOutput")

    with tile.TileContext(nc) as tc:
        with (
            tc.tile_pool(name="const", bufs=1) as constp,
            tc.tile_pool(name="inp", bufs=1) as inp,
            tc.tile_pool(name="hp", bufs=1) as hp,
            tc.tile_pool(name="sp", bufs=2) as sp,
            tc.tile_pool(name="op", bufs=2) as op,
            tc.tile_pool(name="psug", bufs=1, space="PSUM") as psug,
            tc.tile_pool(name="psdn", bufs=1, space="PSUM") as psdn,
        ):
            # zeros tile for PE warmup (DVE memset starts fast; values don't
            # matter for HAM, only PE busy-ness)
            wz = constp.tile([128, 256], BF16, tag="wz")
            nc.vector.memset(wz[:], 0.0)

            # ---- input loads ----
            wsg_sb = inp.tile([128, HC, D], BF16, tag="wsg")
            wsu_sb = inp.tile([128, HC, D], BF16, tag="wsu")
            xs_sb = inp.tile([128, DC, NT], BF16, tag="xs")
            xg_sb = inp.tile([128, DC, CAP], BF16, tag="xg")
            # A single queue gets the full ~360GB/s and drains strictly in
            # order, so the stream is laid out in exact consumption order:
            # sh0 is paced by the xs quarters interleaved with its weights,
            # then each later phase's weights land just ahead of its start.
            # (Two queues split bandwidth round-robin and break the global
            # order -- measured slower.)
            wg_sb = inp.tile([128, HC, D], BF16, tag="wg")
            wu_sb = inp.tile([128, HC, D], BF16, tag="wu")
            wsd_sb = inp.tile([128, HC, D], BF16, tag="wsd")
            wd_sb = inp.tile([128, HC, D], BF16, tag="wd")
            nc.sync.dma_start(wsg_sb[:, 0, :], wsg_d[:, 0, :])
            nc.sync.dma_start(xs_sb[:, 0:2, :], xs_d[:, 0:2, :])
            nc.sync.dma_start(wsu_sb[:, 0, :], wsu_d[:, 0, :])
            nc.sync.dma_start(xs_sb[:, 2:4, :], xs_d[:, 2:4, :])
            nc.sync.dma_start(wsg_sb[:, 1, :], wsg_d[:, 1, :])
            nc.sync.dma_start(xs_sb[:, 4:6, :], xs_d[:, 4:6, :])
            # xs67 ahead of wsu1: sh0's tail matmuls need it before sh1's
            # up chain needs wsu1 (sh1 runs gate-then-up so wsu1 has until
            # sh1's midpoint to land)
            nc.sync.dma_start(xs_sb[:, 6:8, :], xs_d[:, 6:8, :])
            nc.sync.dma_start(wsu_sb[:, 1, :], wsu_d[:, 1, :])
            for hc in range(2, HC):
                nc.sync.dma_start(wsg_sb[:, hc, :], wsg_d[:, hc, :])
                nc.sync.dma_start(wsu_sb[:, hc, :], wsu_d[:, hc, :])
            nc.sync.dma_start(xg_sb[:, 0:4, :], xg_d[:, 0:4, :])
            nc.sync.dma_start(xg_sb[:, 4:8, :], xg_d[:, 4:8, :])
            for hc in range(HC):
                nc.sync.dma_start(wg_sb[:, hc, :], wg_d[:, hc, :])
                nc.sync.dma_start(wu_sb[:, hc, :], wu_d[:, hc, :])
            nc.sync.dma_start(wsd_sb[:], wsd_d[:])
            nc.sync.dma_start(wd_sb[:], wd_d[:])

            # ---- PE p-state warmup: independent MMs alternating two banks
            # (~46% PE duty -- deliberately NOT a dense chain: a 100%-duty
            # warmup trips the P0 power downclock and the whole kernel then
            # runs at 2.0GHz instead of 2.4).  Sized to end right when the
            # first phase's operands land (~12.5us). ----
            for w in range(N_WARM):
                ps_w = psdn.tile([128, 128], F32, tag=("shA" if w % 2 == 0 else "shB"))
                nc.tensor.matmul(ps_w[:], wz[:, 0:128], wz[:, 0:128], start=True, stop=True)

            h_s = hp.tile([128, HC, NT], BF16, tag="hs")
            h_r = hp.tile([128, HC, CAP], BF16, tag="hr")

            # ---- up/gate phases: psum banks rotate over 2 tag-pairs so a
            # phase never waits on the drain of the previous one (phase k's
            # silu/mult run during phase k+1, well before phase k+2 reuses
            # the banks) ----
            def up_gate(x_sb, wgt_sb, wup_sb, n, h, hc, pair, interleave):
                ps_g = psug.tile([128, n], F32, tag=f"g{pair}")
                ps_u = psug.tile([128, n], F32, tag=f"u{pair}")

                def mm_g(dc):
                    nc.tensor.matmul(
                        ps_g[:],
                        wgt_sb[:, hc, dc * 128 : (dc + 1) * 128],
                        x_sb[:, dc, :],
                        start=(dc == 0),
                        stop=(dc == DC - 1),
                    )

                def mm_u(dc):
                    nc.tensor.matmul(
                        ps_u[:],
                        wup_sb[:, hc, dc * 128 : (dc + 1) * 128],
                        x_sb[:, dc, :],
                        start=(dc == 0),
                        stop=(dc == DC - 1),
                    )

                if interleave:
                    for dc in range(DC):
                        mm_g(dc)
                        mm_u(dc)
                else:
                    # gate chain first: the up weights have until mid-phase
                    # to land (used when this phase's wup DMA is still in
                    # flight at phase start)
                    for dc in range(DC):
                        mm_g(dc)
                    for dc in range(DC):
                        mm_u(dc)
                sil = sp.tile([128, n], F32, tag="sil")
                nc.scalar.activation(sil[:], ps_g[:], ACT.Silu)
                nc.vector.tensor_tensor(h[:, hc, :], sil[:], ps_u[:], op=ALU.mult)

            phases = [(xs_sb, wsg_sb, wsu_sb, NT, h_s, hc) for hc in range(HC)]
            phases += [(xg_sb, wg_sb, wu_sb, CAP, h_r, hc) for hc in range(HC)]
            for k, (x_sb, wgt_sb, wup_sb, n, h, hc) in enumerate(phases):
                up_gate(x_sb, wgt_sb, wup_sb, n, h, hc, "AB"[k % 2], k != 1)

            # ---- down projections, interleaved sh/rt per output chunk:
            # 4-bank PSUM double buffer; sh copies on VectorE -> sync DMA
            # FIFO, rt copies on ScalarE -> scalar DMA FIFO ----
            def down_chunk(dc, w_sb, h, n, out_d, par, otag, cp_eng, q_eng):
                ps_d = psdn.tile([128, n], F32, tag=par)
                for hc in range(HC):
                    nc.tensor.matmul(
                        ps_d[:],
                        w_sb[:, hc, dc * 128 : (dc + 1) * 128],
                        h[:, hc, :],
                        start=(hc == 0),
                        stop=(hc == HC - 1),
                    )
                o = op.tile([128, n], BF16, tag=otag, bufs=3)
                if cp_eng == "v":
                    nc.vector.tensor_copy(o[:], ps_d[:])
                else:
                    nc.scalar.activation(o[:], ps_d[:], ACT.Copy)
                q_eng.dma_start(out_d[dc], o[:])

            def down_chunk_split(dc, w_sb, h, n, out_d, otag):
                # final chunk: two independent 256-col psum groups so the
                # first half's copy+DMA overlaps the second half's matmuls,
                # and the two halves drain on both engines/queues
                for hf, (par, cp, q) in enumerate(
                    [("shA", "v", nc.sync), ("shB", "s", nc.scalar)]
                ):
                    cols = slice(hf * (n // 2), (hf + 1) * (n // 2))
                    ps_d = psdn.tile([128, n // 2], F32, tag=par)
                    for hc in range(HC):
                        nc.tensor.matmul(
                            ps_d[:],
                            w_sb[:, hc, dc * 128 : (dc + 1) * 128],
                            h[:, hc, cols],
                            start=(hc == 0),
                            stop=(hc == HC - 1),
                        )
                    o = op.tile([128, n // 2], BF16, tag=f"{otag}{hf}", bufs=2)
                    if cp == "v":
                        nc.vector.tensor_copy(o[:], ps_d[:])
                    else:
                        nc.scalar.activation(o[:], ps_d[:], ACT.Copy)
                    q.dma_start(out_d[dc, :, cols], o[:])

            for dc in range(DC):
                par = "A" if dc % 2 == 0 else "B"
                if dc < DC - 1:
                    down_chunk(dc, wsd_sb, h_s, NT, out_sh, f"sh{par}", "osh", "v", nc.sync)
                    down_chunk(dc, wd_sb, h_r, CAP, out_rt, f"rt{par}", "ort", "s", nc.scalar)
                else:
                    down_chunk_split(dc, wsd_sb, h_s, NT, out_sh, "osh7")
                    down_chunk_split(dc, wd_sb, h_r, CAP, out_rt, "ort7")

    nc.compile()
    return nc


_NC_CACHE = None


def _get_program():
    global _NC_CACHE
    if _NC_CACHE is None:
        _NC_CACHE = build_program()
    return _NC_CACHE


def _xpose_pdc(m):
    """[1024, X] -> [128, 8, X] with row (dc*128+p) at [p, dc]."""
    return np.ascontiguousarray(m.reshape(DC, 128, -1).transpose(1, 0, 2))


def _wlay_upgate(w):
    """[1024(d), 512(de)] -> [128, HC, D]: [p, hc, dc*128+j] = w[dc*128+p, hc*128+j]."""
    return np.ascontiguousarray(
        w.reshape(DC, 128, HC, 128).transpose(1, 2, 0, 3).reshape(128, HC, D)
    )


def _wlay_down(w):
    """[512(de), 1024(d)] -> [128, HC, D]: [p, hc, i] = w[hc*128+p, i]."""
    return np.ascontiguousarray(w.reshape(HC, 128, D).transpose(1, 0, 2))


def _silu(x):
    return x / (1.0 + np.exp(-x))


def kernel(x, W_g, Wg_e, Wu_e, Wd_e, Wg_s, Wu_s, Wd_s, _trace=False, _trace_kwargs=None):
    nc = _get_program()

    xf = np.asarray(x, dtype=np.float32).reshape(2 * 1024, D)
    W_g = np.asarray(W_g, dtype=np.float32)
    Wg_e = np.asarray(Wg_e, dtype=np.float32)
    Wu_e = np.asarray(Wu_e, dtype=np.float32)
    Wd_e = np.asarray(Wd_e, dtype=np.float32)
    Wg_s = np.asarray(Wg_s, dtype=np.float32)
    Wu_s = np.asarray(Wu_s, dtype=np.float32)
    Wd_s = np.asarray(Wd_s, dtype=np.float32)

    # ---- host gating (exact f32; top-2 sets match the jax reference,
    # min top2-top3 prob gap at seed 0 is 6.8e-5 >> f32 matmul noise) ----
    logits = xf @ W_g
    p = np.exp(logits - logits.max(axis=1, keepdims=True))
    p /= p.sum(axis=1, keepdims=True)                      # [N, E] softmax
    top2 = np.argsort(-p, axis=1, kind="stable")[:, :2]    # [N, 2]
    sel = np.zeros((xf.shape[0], E), dtype=bool)
    sel[np.arange(xf.shape[0])[:, None], top2] = True
    toks = [np.nonzero(sel[:, e])[0] for e in range(E)]    # ascending per expert

    # ---- per-core device inputs ----
    in_maps = []
    for c in range(N_CORES):
        e, t, g = c, c // 2, c % 2
        tl = toks[e][:CAP]
        xg = np.zeros((D, CAP), dtype=BF)
        xg[:, : len(tl)] = xf[tl].T.astype(BF)
        in_maps.append(
            {
                "xs": _xpose_pdc(xf[t * NT : (t + 1) * NT].T.astype(BF)),
                "xg": _xpose_pdc(xg),
                "wsg": _wlay_upgate(Wg_s[:, DE * g : DE * (g + 1)].astype(BF)),
                "wsu": _wlay_upgate(Wu_s[:, DE * g : DE * (g + 1)].astype(BF)),
                "wg": _wlay_upgate(Wg_e[e].astype(BF)),
                "wu": _wlay_upgate(Wu_e[e].astype(BF)),
                "wsd": _wlay_down(Wd_s[DE * g : DE * (g + 1), :].astype(BF)),
                "wd": _wlay_down(Wd_e[e].astype(BF)),
            }
        )

    res = run_bass_kernel_spmd(
        nc, in_maps, list(range(N_CORES)), trace=_trace, **(_trace_kwargs or {})
    )

    # ---- host combine (unshard) ----
    out = np.empty((2 * 1024, D), dtype=np.float32)
    for t in range(4):
        sh = np.asarray(res.results[2 * t]["out_sh"], dtype=np.float32).reshape(D, NT)
        sh += np.asarray(res.results[2 * t + 1]["out_sh"], dtype=np.float32).reshape(
            D, NT
        )
        out[t * NT : (t + 1) * NT, :] = sh.T
    for e in range(E):
        tl = toks[e]
        nd = min(len(tl), CAP)
        rt = np.asarray(res.results[e]["out_rt"], dtype=np.float32).reshape(D, CAP)
        out[tl[:nd]] += (rt[:, :nd] * p[tl[:nd], e][None, :]).T
        if len(tl) > CAP:  # over-capacity tokens: exact host fallback
            to = tl[CAP:]
            xo = xf[to]
            hh = _silu(xo @ Wg_e[e]) * (xo @ Wu_e[e])
            out[to] += (hh @ Wd_e[e]) * p[to, e][:, None]

    result = out.reshape(2, 1024, D)
    if _trace:
        return result, res
    return result
